# revision 2
# baseline (speedup 1.0000x reference)
"""Masked (sparse) attention for Trainium2 — Bass/Tile kernel, 8 NeuronCores.

Problem (per batch element b of 8):
  S = Q K^T / 8;  S[mask==0] = -1e10;  out = softmax_m(S) @ V
  Q:[n,d] K:[m,d] V:[m,dv] mask:[n,m]  (n=m=4096, d=dv=64, fp32)

Sharding: batch-parallel — one batch element per NeuronCore (8 cores).

Per-core algorithm (engine-balanced):
  * Q^T/K^T staged once via PE transposes (fp16 — exact enough, lets the PE
    stream at 1 cycle/row with fast weight loads).
  * S' = Q K^T + BIG*mask accumulated in PSUM.  The mask is folded in two
    ways, split 3:2 between engines to balance occupancy:
      - PE path: identity-matmul  S' += (BIG*I).T @ mask_bf16  where the
        moving operand is the bf16 high-half view of the fp32 mask in SBUF
        (fp32 0.0/1.0 truncate to bf16 0.0/1.0 exactly — no cast pass).
      - DVE path: in-place scalar_tensor_tensor  S' = mask*BIG + S' on PSUM.
  * P = exp(S'/8 - BIG/8) on ScalarE (PSUM -> SBUF fp16); masked entries
    become e^-16 ~ 1e-7 (negligible vs denominators ~2e3).  Row-sums of P
    ride along in the activation's accum_out (fp32) - free denominators.
  * P^T via PE transposes (fp16, 1 cyc/row) -> PSUM -> DVE copy to SBUF
    (2x packed mode), then PV matmuls accumulate out in PSUM.
  * out = PV * (1/rowsum) on DVE, single batched DMA out.
"""

import os
import sys

import numpy as np

for _p in ("/opt/trn_rl_repo", "/root/.axon_site/_ro/trn_rl_repo"):
    if os.path.isdir(_p) and _p not in sys.path:
        sys.path.append(_p)

from contextlib import ExitStack

import concourse.bass as bass  # noqa: E402
import concourse.mybir as mybir  # noqa: E402
import concourse.tile as tile  # noqa: E402
from concourse import bacc  # noqa: E402
from concourse.bass_utils import run_bass_kernel_spmd  # noqa: E402
from concourse.masks import make_identity  # noqa: E402

FP32 = mybir.dt.float32
BF16 = mybir.dt.bfloat16
FP16 = mybir.dt.float16
AF = mybir.ActivationFunctionType
ALU = mybir.AluOpType
AXL = mybir.AxisListType

BIG = 128.0          # additive mask offset; exp bias = -BIG/8
N_CORES = 8
B, N, M, D, DV = 8, 4096, 4096, 64, 64


def _build(n=N, m=M, d=D, dv=DV, reps=1, n_cores=N_CORES, chunk=512,
           mask_tile=2048, dve_mask_num=2, dve_mask_den=5, mask_bufs=6,
           p_bufs=3):
    lp = FP16
    mask_tile = min(mask_tile, m)
    NB = n // 128
    MB = m // 128
    MT = m // mask_tile
    assert n % 128 == 0 and m % mask_tile == 0 and mask_tile % chunk == 0
    assert chunk % 128 == 0 and chunk <= 512

    nc = bacc.Bacc("TRN2", target_bir_lowering=False, debug=False,
                   num_devices=n_cores)

    q_d = nc.dram_tensor("queries", [n, d], FP32, kind="ExternalInput").ap()
    k_d = nc.dram_tensor("keys", [m, d], FP32, kind="ExternalInput").ap()
    v_d = nc.dram_tensor("values", [m, dv], FP32, kind="ExternalInput").ap()
    mask_d = nc.dram_tensor("visible_masking", [n, m], FP32,
                            kind="ExternalInput").ap()
    out_d = nc.dram_tensor("out", [n, dv], FP32, kind="ExternalOutput").ap()

    q_dv = q_d.rearrange("(b p) x -> p b x", p=128)
    k_dv = k_d.rearrange("(b p) x -> p b x", p=128)
    v_dv = v_d.rearrange("(b p) x -> p b x", p=128)
    out_dv = out_d.rearrange("(b p) x -> p b x", p=128)

    with tile.TileContext(nc) as tc, ExitStack() as ctx:
        per = ctx.enter_context(tc.tile_pool(name="persist", bufs=1))
        ident_lp = per.tile([128, 128], lp)
        make_identity(nc, ident_lp)
        big_i_bf = per.tile([128, 128], BF16)
        make_identity(nc, big_i_bf)
        nc.vector.tensor_scalar_mul(big_i_bf[:], big_i_bf[:], BIG)
        ident_f32 = per.tile([128, 128], FP32)
        make_identity(nc, ident_f32)
        exp_bias = per.tile([128, 1], FP32)
        nc.vector.memset(exp_bias[:], -BIG / 8.0)
        qt_sb = per.tile([d, n], lp)
        kt_sb = per.tile([d, m], lp)
        v_sb = per.tile([128, MB * dv], lp)
        out_acc = per.tile([128, NB * dv], FP32)

        # setup: batched loads, PE transposes for Q^T/K^T, V cast
        with tc.tile_pool(name="setup", bufs=2) as sp, \
             tc.tile_pool(name="setup_ps", bufs=4, space="PSUM") as spp:
            q_all = sp.tile([128, NB * d], FP32, tag="qk")
            nc.sync.dma_start(q_all[:].rearrange("p (b x) -> p b x", x=d),
                              q_dv)
            k_all = sp.tile([128, MB * d], FP32, tag="qk")
            nc.sync.dma_start(k_all[:].rearrange("p (b x) -> p b x", x=d),
                              k_dv)
            v_all = sp.tile([128, MB * dv], FP32, tag="qk")
            nc.sync.dma_start(v_all[:].rearrange("p (b x) -> p b x", x=dv),
                              v_dv)
            nc.vector.tensor_copy(v_sb[:], v_all[:])
            for nb in range(NB):
                qt_ps = spp.tile([d, 128], FP32, tag="tp")
                nc.tensor.transpose(qt_ps[:], q_all[:, nb * d:(nb + 1) * d],
                                    ident_f32[:])
                nc.vector.tensor_copy(qt_sb[:, nb * 128:(nb + 1) * 128],
                                      qt_ps[:])
            for mb in range(MB):
                kt_ps = spp.tile([d, 128], FP32, tag="tp")
                nc.tensor.transpose(kt_ps[:], k_all[:, mb * d:(mb + 1) * d],
                                    ident_f32[:])
                nc.vector.tensor_copy(kt_sb[:, mb * 128:(mb + 1) * 128],
                                      kt_ps[:])

        mp = ctx.enter_context(tc.tile_pool(name="maskp", bufs=mask_bufs))
        bp = ctx.enter_context(tc.tile_pool(name="bfp", bufs=p_bufs))
        sps = ctx.enter_context(tc.tile_pool(name="spsum", bufs=2,
                                             space="PSUM"))
        pps = ctx.enter_context(tc.tile_pool(name="ptpsum", bufs=2,
                                             space="PSUM"))
        ops_ = ctx.enter_context(tc.tile_pool(name="opsum", bufs=2,
                                              space="PSUM"))
        fp = ctx.enter_context(tc.tile_pool(name="finp", bufs=2))

        group = min(2 * chunk, m)
        NG = m // group
        SPG = group // chunk
        GB = group // 128

        for _ in range(reps):
            for nb in range(NB):
                no = nb * 128
                out_ps = ops_.tile([128, dv], FP32, tag="acc")
                denom = fp.tile([128, NG], FP32, tag="denom")
                mtiles = []
                for mt in range(MT):
                    mask_t = mp.tile([128, mask_tile], FP32, tag="mask")
                    nc.sync.dma_start(
                        mask_t[:],
                        mask_d[no:no + 128,
                               mt * mask_tile:(mt + 1) * mask_tile])
                    mtiles.append(mask_t)

                for g in range(NG):
                    go = g * group
                    on_dve = ((nb * NG + g) % dve_mask_den) < dve_mask_num
                    s_ps = sps.tile([128, group], FP32, tag="s")
                    # same-weight matmuls adjacent: both S sub-matmuls share
                    # Q^T as the stationary operand, both mask-adds share
                    # BIG*I -- fewer PE weight reloads.
                    for s in range(SPG):
                        mo = go + s * chunk
                        so = s * chunk
                        nc.tensor.matmul(
                            s_ps[:, so:so + chunk],
                            lhsT=qt_sb[:, no:no + 128],
                            rhs=kt_sb[:, mo:mo + chunk],
                            start=True, stop=on_dve)
                    if not on_dve:
                        for s in range(SPG):
                            mo = go + s * chunk
                            so = s * chunk
                            mview = (mtiles[mo // mask_tile][:].bitcast(BF16)
                                     .rearrange("p (x two) -> p x two",
                                                two=2))
                            off = mo % mask_tile
                            nc.tensor.matmul(
                                s_ps[:, so:so + chunk],
                                lhsT=big_i_bf[:],
                                rhs=mview[:, off:off + chunk, 1],
                                start=False, stop=True)
                    if on_dve:
                        mt_i = go // mask_tile
                        off = go % mask_tile
                        nc.vector.scalar_tensor_tensor(
                            out=s_ps[:],
                            in0=mtiles[mt_i][:, off:off + group],
                            scalar=BIG,
                            in1=s_ps[:],
                            op0=ALU.mult, op1=ALU.add)

                    p_sb = bp.tile([128, group], lp, tag="p")
                    nc.scalar.activation(p_sb[:], s_ps[:], AF.Exp,
                                         bias=exp_bias[:], scale=1.0 / 8.0,
                                         accum_out=denom[:, g:g + 1])

                    pt_ps = pps.tile([128, group], lp, tag="pt")
                    for k in range(GB):
                        nc.tensor.transpose(
                            pt_ps[:, k * 128:(k + 1) * 128],
                            p_sb[:, k * 128:(k + 1) * 128],
                            ident_lp[:])
                    pt_sb = bp.tile([128, group], lp, tag="ptsb")
                    nc.vector.tensor_copy(pt_sb[:], pt_ps[:])
                    for k in range(GB):
                        mb = g * GB + k
                        nc.tensor.matmul(
                            out_ps[:],
                            lhsT=pt_sb[:, k * 128:(k + 1) * 128],
                            rhs=v_sb[:, mb * dv:(mb + 1) * dv],
                            start=(mb == 0), stop=(mb == MB - 1))

                dsum = fp.tile([128, 1], FP32, tag="dsum")
                nc.vector.tensor_reduce(dsum[:], denom[:], AXL.X, ALU.add)
                recip = fp.tile([128, 1], FP32, tag="recip")
                nc.vector.reciprocal(recip[:], dsum[:])
                nc.vector.tensor_scalar(out_acc[:, nb * dv:(nb + 1) * dv],
                                        out_ps[:], recip[:], None, ALU.mult)
            nc.sync.dma_start(out_dv,
                              out_acc[:].rearrange("p (b x) -> p b x", x=dv))

    nc.compile()
    return nc


_CACHE = {}


def _get_nc(reps=1):
    key = ("nc", reps)
    if key not in _CACHE:
        _CACHE[key] = _build(reps=reps)
    return _CACHE[key]


def kernel(queries, keys, values, visible_masking):
    """Full inputs [8, 4096, ...] -> full output [8, 4096, 64] (fp32)."""
    queries = np.ascontiguousarray(np.asarray(queries, dtype=np.float32))
    keys = np.ascontiguousarray(np.asarray(keys, dtype=np.float32))
    values = np.ascontiguousarray(np.asarray(values, dtype=np.float32))
    visible_masking = np.ascontiguousarray(
        np.asarray(visible_masking, dtype=np.float32))
    assert queries.shape == (B, N, D), queries.shape

    nc = _get_nc()
    in_maps = [{
        "queries": queries[c],
        "keys": keys[c],
        "values": values[c],
        "visible_masking": visible_masking[c],
    } for c in range(N_CORES)]
    res = run_bass_kernel_spmd(nc, in_maps, core_ids=list(range(N_CORES)))
    return np.stack([res.results[c]["out"] for c in range(N_CORES)], axis=0)
